# revision 4
# baseline (speedup 1.0000x reference)
"""DrBCRNN (graph message-passing GRU) Trainium2 kernel, 8-way node-sharded.

Per rep (x5):
    pool = segment_sum(h[src] * edge_w, dst)        # sparse gather + scatter-add
    gx   = pool @ (W_node @ W_gru) + (b_node @ W_gru + b_in)   # fused on host
    z    = sigmoid(gx_z + b_rec_z); hh = tanh(gx_h + r*b_rec_h)
    h    = l2_normalize((1-z) * hh)

Device mapping (per core; nodes sharded 8 ways by dst):
    - edges sorted by (dst tile of 128 nodes, src-quarter), padded to 128-edge
      chunks with per-(tile,quarter) chunk counts shared across cores (SPMD:
      one instruction stream for all 8 cores).
    - gather: one gpsimd dma_gather per (supertile of 4 tiles, src quarter)
      pulls h[src] rows into SBUF [128e, chunks, 128u]. Indices are int16
      local to a <=32K-row quarter slice of the table, pre-wrapped on the host
      into the ucode's [16, n/16] column-major layout.
    - scatter-add: per chunk, one-hot selector S[e,d] = w_e * (dst_e == d)
      via one fused DVE tensor_scalar (is_equal, mult) against an iota row;
      PE matmul G.T @ S accumulates pool_T[u, d] in PSUM.
    - GRU: two 128x128 matmuls (W_eff z/h chunks; r path only when b_rec_h
      is nonzero) + ACT sigmoid/tanh + DVE mul.
    - l2-normalize: PE transpose to node-major, ACT square w/ row-sum
      accumulator, sqrt, DVE reciprocal + scale.
    - h slices are AllGathered across the 8 cores between reps so every core
      has the full gather table.
"""

import sys

sys.path.insert(0, "/opt/trn_rl_repo")

from contextlib import ExitStack

import numpy as np

import concourse.bass as bass
import concourse.tile as tile
from concourse import bacc, mybir
from concourse import bass_utils
from concourse.library_config import mlp

P = 128
F32 = mybir.dt.float32
I16 = mybir.dt.int16

N_FULL = 100000
UNITS = 128
REPS = 5
NCORES = 8
ST_TILES = 4          # tiles per gather supertile
QMAX = 32700          # max rows addressable by int16 gather indices

TRACE = False
LAST_RESULT = None

_CACHE = {}


def _preprocess_edges(src, dst, edge_w, n, ncores):
    """Sort/pad edges by (dst tile, src quarter); build the per-core device
    arrays and the shared (SPMD-identical) chunk/gather schedule."""
    assert n % ncores == 0, (n, ncores)
    nloc_raw = n // ncores
    tiles = (nloc_raw + P - 1) // P
    nloc = tiles * P
    npad = nloc * ncores

    nq = max(1, -(-npad // QMAX))
    qrows = -(-npad // nq)  # rows per quarter slice
    assert qrows <= 32767

    e = src.shape[0]
    core = dst // nloc_raw
    local = dst - core * nloc_raw
    tile_g = local // P
    dl = (local % P).astype(np.float32)
    src_pad = ((src // nloc_raw) * nloc + (src % nloc_raw)).astype(np.int64)
    q = src_pad // qrows
    idx_local = (src_pad - q * qrows).astype(np.int32)

    # bucket = (core, tile, quarter)
    bucket = (core * tiles + tile_g) * nq + q
    order = np.argsort(bucket, kind="stable")
    s_il = idx_local[order]
    s_dl = dl[order]
    s_w = edge_w[order].astype(np.float32)
    s_bk = bucket[order]

    counts = np.bincount(bucket, minlength=ncores * tiles * nq).reshape(
        ncores, tiles, nq
    )
    # chunks per (tile, quarter), shared across cores
    c_tq = -(-counts.max(axis=0) // P)  # [tiles, nq]

    n_st = -(-tiles // ST_TILES)
    # column layout: for st: for q: for t in st: C(t,q) chunks
    colbase = np.zeros((tiles, nq), np.int64)
    ops = []  # (st, q, col_start, n_chunks, idx16_col_off)
    cur = 0
    i16off = 0
    for st in range(n_st):
        t0, t1 = st * ST_TILES, min((st + 1) * ST_TILES, tiles)
        for qq in range(nq):
            op_start = cur
            for t in range(t0, t1):
                colbase[t, qq] = cur
                cur += int(c_tq[t, qq])
            nck = cur - op_start
            if nck > 0:
                ops.append((st, qq, int(op_start), int(nck), int(i16off)))
                i16off += 8 * nck  # n/16 columns = 128*nck/16
    t_total = int(cur)
    i16_total = int(i16off)

    # per-tile ordered chunk column list
    tile_cols = []
    for t in range(tiles):
        cols = []
        for qq in range(nq):
            cols.extend(range(int(colbase[t, qq]), int(colbase[t, qq] + c_tq[t, qq])))
        tile_cols.append(cols)
    # supertile of each column (for G-tile-relative addressing)
    st_colbase = [int(colbase[st * ST_TILES, 0]) for st in range(n_st)]
    st_ncols = [
        (st_colbase[st + 1] if st + 1 < n_st else t_total) - st_colbase[st]
        for st in range(n_st)
    ]

    # edge placement
    flat_counts = counts.reshape(-1)
    group_start = np.zeros(flat_counts.size, np.int64)
    group_start[1:] = np.cumsum(flat_counts)[:-1]
    pos = np.arange(e, dtype=np.int64) - np.repeat(group_start, flat_counts)

    e_core = s_bk // (tiles * nq)
    e_tile = (s_bk // nq) % tiles
    e_q = s_bk % nq
    col = colbase[e_tile, e_q] + pos // P
    row = pos % P

    edl = np.zeros((ncores, P, t_total), np.float32)
    ew = np.zeros((ncores, P, t_total), np.float32)
    edl[e_core, row, col] = s_dl
    ew[e_core, row, col] = s_w

    # idx16 wrapped layout, per op: seq j = (col - op_start)*128 + row;
    # wrapped position [j % 16, i16_off + j // 16]
    idx16 = np.zeros((ncores, 16, i16_total), np.int16)
    opstart_of_col = np.zeros(t_total, np.int64)
    i16off_of_col = np.zeros(t_total, np.int64)
    for (_st, _q, cstart, nck, ioff) in ops:
        opstart_of_col[cstart : cstart + nck] = cstart
        i16off_of_col[cstart : cstart + nck] = ioff
    j = (col - opstart_of_col[col]) * P + row
    idx16[e_core, j % 16, i16off_of_col[col] + j // 16] = s_il.astype(np.int16)
    idx16 = np.tile(idx16, (1, 8, 1))  # [ncores, 128, i16_total]

    return dict(
        nloc_raw=nloc_raw,
        tiles=tiles,
        nloc=nloc,
        npad=npad,
        nq=nq,
        qrows=qrows,
        t_total=t_total,
        i16_total=i16_total,
        ops=tuple(ops),
        tile_cols=tuple(tuple(c) for c in tile_cols),
        st_colbase=tuple(st_colbase),
        st_ncols=tuple(st_ncols),
        edl=edl,
        ew=ew,
        idx16=idx16,
    )


def _build_program(cfg):
    tiles = cfg["tiles"]
    nloc = cfg["nloc"]
    npad = cfg["npad"]
    nq = cfg["nq"]
    qrows = cfg["qrows"]
    t_total = cfg["t_total"]
    i16_total = cfg["i16_total"]
    ops = cfg["ops"]
    tile_cols = cfg["tile_cols"]
    st_colbase = cfg["st_colbase"]
    st_ncols = cfg["st_ncols"]
    reps = cfg["reps"]
    ncores = cfg["ncores"]
    full_gru = cfg["full_gru"]
    use_bz = cfg["use_bz"]
    use_bh = cfg["use_bh"]
    n_st = len(st_colbase)
    cmax_st = max(st_ncols)

    nc = bacc.Bacc("TRN2", target_bir_lowering=False, debug=False, num_devices=ncores)

    msg_d = nc.dram_tensor("msg", [npad, P], F32, kind="ExternalInput")
    idx16_d = nc.dram_tensor("idx16", [P, i16_total], I16, kind="ExternalInput")
    edl_d = nc.dram_tensor("edl", [P, t_total], F32, kind="ExternalInput")
    ew_d = nc.dram_tensor("ew", [P, t_total], F32, kind="ExternalInput")
    weffz_d = nc.dram_tensor("weffz", [P, P], F32, kind="ExternalInput")
    weffh_d = nc.dram_tensor("weffh", [P, P], F32, kind="ExternalInput")
    iota_d = nc.dram_tensor("iota", [P, P], F32, kind="ExternalInput")
    ident_d = nc.dram_tensor("ident", [P, P], F32, kind="ExternalInput")
    nbz_d = nc.dram_tensor("nbz", [P, 1], F32, kind="ExternalInput")
    bhin_d = nc.dram_tensor("bhin", [P, 1], F32, kind="ExternalInput")
    if full_gru:
        weffr_d = nc.dram_tensor("weffr", [P, P], F32, kind="ExternalInput")
        br_d = nc.dram_tensor("br", [P, 1], F32, kind="ExternalInput")
        bhrec_d = nc.dram_tensor("bhrec", [P, 1], F32, kind="ExternalInput")

    out_d = nc.dram_tensor("out", [reps, nloc, P], F32, kind="ExternalOutput")

    n_agbuf = min(2, reps - 1) if reps > 1 else 0
    ag_in = [
        nc.dram_tensor(f"ag_in{i}", [nloc, P], F32, kind="Internal")
        for i in range(n_agbuf)
    ]
    h_full = [
        nc.dram_tensor(
            f"h_full{r}", [npad, P], F32, kind="Internal", addr_space="Shared"
        )
        for r in range(reps - 1)
    ]

    with tile.TileContext(nc) as tc, ExitStack() as ctx:
        const = ctx.enter_context(tc.tile_pool(name="const", bufs=1))
        gpool = ctx.enter_context(tc.tile_pool(name="g", bufs=2))
        selp = ctx.enter_context(tc.tile_pool(name="sel", bufs=4))
        work = ctx.enter_context(tc.tile_pool(name="work", bufs=2))
        psp = ctx.enter_context(tc.tile_pool(name="ps", bufs=2, space="PSUM"))

        nc.gpsimd.load_library(mlp)

        idx16_sb = const.tile([P, i16_total], I16)
        nc.sync.dma_start(idx16_sb[:], idx16_d[:])
        edl_sb = const.tile([P, t_total], F32)
        nc.sync.dma_start(edl_sb[:], edl_d[:])
        ew_sb = const.tile([P, t_total], F32)
        nc.sync.dma_start(ew_sb[:], ew_d[:])
        iota_sb = const.tile([P, P], F32)
        nc.sync.dma_start(iota_sb[:], iota_d[:])
        ident_sb = const.tile([P, P], F32)
        nc.sync.dma_start(ident_sb[:], ident_d[:])
        weffz_sb = const.tile([P, P], F32)
        nc.sync.dma_start(weffz_sb[:], weffz_d[:])
        weffh_sb = const.tile([P, P], F32)
        nc.sync.dma_start(weffh_sb[:], weffh_d[:])
        nbz_sb = const.tile([P, 1], F32)
        nc.sync.dma_start(nbz_sb[:], nbz_d[:])
        bhin_sb = const.tile([P, 1], F32)
        nc.sync.dma_start(bhin_sb[:], bhin_d[:])
        if full_gru:
            weffr_sb = const.tile([P, P], F32)
            nc.sync.dma_start(weffr_sb[:], weffr_d[:])
            br_sb = const.tile([P, 1], F32)
            nc.sync.dma_start(br_sb[:], br_d[:])
            bhrec_sb = const.tile([P, 1], F32)
            nc.sync.dma_start(bhrec_sb[:], bhrec_d[:])

        ngx = 3 if full_gru else 2

        for r in range(reps):
            table = msg_d if r == 0 else h_full[r - 1]
            for st in range(n_st):
                scb = st_colbase[st]
                g_sb = gpool.tile([P, cmax_st, P], F32, tag="G")
                for (op_st, qq, cstart, nck, ioff) in ops:
                    if op_st != st:
                        continue
                    qa = qq * qrows
                    qb = min(qa + qrows, npad)
                    nc.gpsimd.dma_gather(
                        g_sb[:, cstart - scb : cstart - scb + nck, :],
                        table[qa:qb, :],
                        idx16_sb[:, ioff : ioff + 8 * nck],
                        nck * P,
                        nck * P,
                        P,
                        single_packet=False,
                    )
                t0, t1 = st * ST_TILES, min((st + 1) * ST_TILES, tiles)
                for t in range(t0, t1):
                    cols = tile_cols[t]
                    pool_ps = psp.tile([P, P], F32, tag="pool")
                    for k, colg in enumerate(cols):
                        sel_sb = selp.tile([P, P], F32, tag="sel")
                        nc.vector.tensor_scalar(
                            out=sel_sb[:],
                            in0=iota_sb[:],
                            scalar1=edl_sb[:, colg : colg + 1],
                            scalar2=ew_sb[:, colg : colg + 1],
                            op0=mybir.AluOpType.is_equal,
                            op1=mybir.AluOpType.mult,
                        )
                        nc.tensor.matmul(
                            out=pool_ps[:],
                            lhsT=g_sb[:, colg - scb, :],
                            rhs=sel_sb[:],
                            start=(k == 0),
                            stop=(k == len(cols) - 1),
                        )

                    pool_sb = work.tile([P, P], F32, tag="poolsb")
                    nc.scalar.copy(pool_sb[:], pool_ps[:])

                    gx_ps = psp.tile([P, ngx * P], F32, tag="gx")
                    nc.tensor.matmul(
                        out=gx_ps[:, 0:P], lhsT=weffz_sb[:], rhs=pool_sb[:],
                        start=True, stop=True,
                    )
                    nc.tensor.matmul(
                        out=gx_ps[:, P : 2 * P], lhsT=weffh_sb[:], rhs=pool_sb[:],
                        start=True, stop=True,
                    )
                    if full_gru:
                        nc.tensor.matmul(
                            out=gx_ps[:, 2 * P : 3 * P], lhsT=weffr_sb[:],
                            rhs=pool_sb[:], start=True, stop=True,
                        )

                    omz_sb = work.tile([P, P], F32, tag="omz")
                    nc.scalar.activation(
                        out=omz_sb[:],
                        in_=gx_ps[:, 0:P],
                        func=mybir.ActivationFunctionType.Sigmoid,
                        bias=nbz_sb[:, :1] if use_bz else 0.0,
                        scale=-1.0,
                    )
                    hh_sb = work.tile([P, P], F32, tag="hh")
                    if full_gru:
                        r_sb = work.tile([P, P], F32, tag="r")
                        nc.scalar.activation(
                            out=r_sb[:],
                            in_=gx_ps[:, 2 * P : 3 * P],
                            func=mybir.ActivationFunctionType.Sigmoid,
                            bias=br_sb[:, :1],
                        )
                        rb_sb = work.tile([P, P], F32, tag="rb")
                        nc.vector.tensor_scalar(
                            out=rb_sb[:], in0=r_sb[:], scalar1=bhrec_sb[:, :1],
                            scalar2=None, op0=mybir.AluOpType.mult,
                        )
                        s_sb = work.tile([P, P], F32, tag="s")
                        nc.vector.tensor_tensor(
                            out=s_sb[:], in0=gx_ps[:, P : 2 * P], in1=rb_sb[:],
                            op=mybir.AluOpType.add,
                        )
                        nc.scalar.activation(
                            out=hh_sb[:], in_=s_sb[:],
                            func=mybir.ActivationFunctionType.Tanh,
                            bias=bhin_sb[:, :1] if use_bh else 0.0,
                        )
                    else:
                        nc.scalar.activation(
                            out=hh_sb[:], in_=gx_ps[:, P : 2 * P],
                            func=mybir.ActivationFunctionType.Tanh,
                            bias=bhin_sb[:, :1] if use_bh else 0.0,
                        )

                    comb_sb = work.tile([P, P], F32, tag="comb")
                    nc.vector.tensor_tensor(
                        out=comb_sb[:], in0=omz_sb[:], in1=hh_sb[:],
                        op=mybir.AluOpType.mult,
                    )

                    tr_ps = psp.tile([P, P], F32, tag="tr")
                    nc.tensor.transpose(
                        out=tr_ps[:], in_=comb_sb[:], identity=ident_sb[:]
                    )
                    sq_sb = work.tile([P, P], F32, tag="sq")
                    ss_sb = work.tile([P, 1], F32, tag="ss")
                    nc.scalar.activation(
                        out=sq_sb[:], in_=tr_ps[:],
                        func=mybir.ActivationFunctionType.Square,
                        accum_out=ss_sb[:],
                    )
                    nc.vector.tensor_scalar(
                        out=ss_sb[:], in0=ss_sb[:], scalar1=1e-12, scalar2=None,
                        op0=mybir.AluOpType.max,
                    )
                    nrm_sb = work.tile([P, 1], F32, tag="nrm")
                    nc.scalar.activation(
                        out=nrm_sb[:], in_=ss_sb[:],
                        func=mybir.ActivationFunctionType.Sqrt,
                    )
                    rn_sb = work.tile([P, 1], F32, tag="rn")
                    nc.vector.reciprocal(rn_sb[:], nrm_sb[:])
                    h_sb = work.tile([P, P], F32, tag="h")
                    nc.vector.tensor_scalar(
                        out=h_sb[:], in0=tr_ps[:], scalar1=rn_sb[:, :1],
                        scalar2=None, op0=mybir.AluOpType.mult,
                    )

                    nc.sync.dma_start(out_d[r, t * P : (t + 1) * P, :], h_sb[:])
                    if r < reps - 1:
                        nc.sync.dma_start(
                            ag_in[r % 2][t * P : (t + 1) * P, :], h_sb[:]
                        )
            if r < reps - 1:
                nc.gpsimd.collective_compute(
                    "AllGather",
                    mybir.AluOpType.bypass,
                    replica_groups=[list(range(ncores))],
                    ins=[ag_in[r % 2][:]],
                    outs=[h_full[r][:]],
                )

    nc.compile()
    return nc


def _prepare(message, src, dst, edge_w, W_node, b_node, W_gru, U_gru, b_in, b_rec,
             n, ncores, reps):
    pre = _preprocess_edges(
        np.asarray(src), np.asarray(dst), np.asarray(edge_w), n, ncores
    )
    nloc_raw, nloc, npad = pre["nloc_raw"], pre["nloc"], pre["npad"]

    W_node64 = np.asarray(W_node, np.float64)
    W_gru64 = np.asarray(W_gru, np.float64)
    W_eff = (W_node64 @ W_gru64).astype(np.float32)
    b_in_eff = (
        np.asarray(b_node, np.float64) @ W_gru64 + np.asarray(b_in, np.float64)
    ).astype(np.float32)
    u = W_eff.shape[0]
    assert u == P
    b_rec = np.asarray(b_rec, np.float32)
    bz = b_in_eff[0:u] + b_rec[0:u]
    br = b_in_eff[u : 2 * u] + b_rec[u : 2 * u]
    bh_in = b_in_eff[2 * u : 3 * u]
    bh_rec = b_rec[2 * u : 3 * u]
    full_gru = bool(np.any(bh_rec != 0.0))
    use_bz = bool(np.any(bz != 0.0))
    use_bh = bool(np.any(bh_in != 0.0))

    msg = np.asarray(message, np.float32)
    msg_pad = np.zeros((npad, P), np.float32)
    msg_pad.reshape(ncores, nloc, P)[:, :nloc_raw, :] = msg.reshape(
        ncores, nloc_raw, P
    )

    iota = np.tile(np.arange(P, dtype=np.float32), (P, 1))
    ident = np.eye(P, dtype=np.float32)

    cfg = dict(
        tiles=pre["tiles"],
        nloc=nloc,
        npad=npad,
        nq=pre["nq"],
        qrows=pre["qrows"],
        t_total=pre["t_total"],
        i16_total=pre["i16_total"],
        ops=pre["ops"],
        tile_cols=pre["tile_cols"],
        st_colbase=pre["st_colbase"],
        st_ncols=pre["st_ncols"],
        reps=reps,
        ncores=ncores,
        full_gru=full_gru,
        use_bz=use_bz,
        use_bh=use_bh,
    )

    in_maps = []
    for c in range(ncores):
        m = dict(
            msg=msg_pad,
            idx16=np.ascontiguousarray(pre["idx16"][c]),
            edl=np.ascontiguousarray(pre["edl"][c]),
            ew=np.ascontiguousarray(pre["ew"][c]),
            weffz=np.ascontiguousarray(W_eff[:, 0:P]),
            weffh=np.ascontiguousarray(W_eff[:, 2 * P : 3 * P]),
            iota=iota,
            ident=ident,
            nbz=(-bz).reshape(P, 1).astype(np.float32),
            bhin=bh_in.reshape(P, 1).astype(np.float32),
        )
        if full_gru:
            m["weffr"] = np.ascontiguousarray(W_eff[:, P : 2 * P])
            m["br"] = br.reshape(P, 1).astype(np.float32)
            m["bhrec"] = bh_rec.reshape(P, 1).astype(np.float32)
        in_maps.append(m)

    return cfg, in_maps, pre


def _assemble_output(res_list, cfg, n, reps):
    # Match the reference faithfully: concat states along features then
    # reshape [n, UNITS, REPS] (an interleaving reshape, NOT a stack):
    # out[n, i, j] = states[(i*reps + j) // P][n, (i*reps + j) % P]
    ncores = cfg["ncores"]
    nloc_raw = n // ncores
    parts = []
    for c in range(ncores):
        buf = res_list[c]["out"]  # [reps, nloc, P]
        parts.append(
            np.transpose(buf[:, :nloc_raw, :], (1, 0, 2)).reshape(
                nloc_raw, P, reps
            )
        )
    return np.ascontiguousarray(np.concatenate(parts, axis=0), dtype=np.float32)


def kernel(message, src, dst, edge_w, W_node, b_node, W_gru, U_gru, b_in, b_rec):
    global LAST_RESULT
    message = np.asarray(message)
    n = message.shape[0]
    reps = REPS
    ncores = NCORES

    cfg, in_maps, _pre = _prepare(
        message, src, dst, edge_w, W_node, b_node, W_gru, U_gru, b_in, b_rec,
        n, ncores, reps,
    )

    key = (n, cfg["t_total"], cfg["ops"], cfg["full_gru"], cfg["use_bz"],
           cfg["use_bh"], reps, ncores)
    nc = _CACHE.get(key)
    if nc is None:
        nc = _build_program(cfg)
        _CACHE[key] = nc

    res = bass_utils.run_bass_kernel_spmd(
        nc, in_maps, core_ids=list(range(ncores)), trace=TRACE
    )
    LAST_RESULT = res
    return _assemble_output(res.results, cfg, n, reps)


# revision 7
# speedup vs baseline: 125.4280x; 125.4280x over previous
"""DrBCRNN (graph message-passing GRU) Trainium2 kernel, 8-way node-sharded.

Per rep (x5):
    pool = segment_sum(h[src] * edge_w, dst)        # sparse gather + scatter-add
    gx   = pool @ (W_node @ W_gru) + (b_node @ W_gru + b_in)   # fused on host
    z    = sigmoid(gx_z + b_rec_z); hh = tanh(gx_h + r*b_rec_h)
    h    = l2_normalize((1-z) * hh)

Device mapping (per core; nodes sharded 8 ways by dst):
    - edges sorted by (dst tile of 128 nodes, src-quarter), padded to 128-edge
      chunks with per-(tile,quarter) chunk counts shared across cores (SPMD:
      one instruction stream for all 8 cores).
    - gather: one gpsimd dma_gather per (supertile of 4 tiles, src quarter)
      pulls h[src] rows into SBUF [128e, chunks, 128u]. Indices are int16
      local to a <=32K-row quarter slice of the table, pre-wrapped on the host
      into the ucode's [16, n/16] column-major layout.
    - scatter-add: per chunk, one-hot selector S[e,d] = w_e * (dst_e == d)
      via one fused DVE tensor_scalar (is_equal, mult) against an iota row;
      PE matmul G.T @ S accumulates pool_T[u, d] in PSUM.
    - GRU: two 128x128 matmuls (W_eff z/h chunks; r path only when b_rec_h
      is nonzero) + ACT sigmoid/tanh + DVE mul.
    - l2-normalize: PE transpose to node-major, ACT square w/ row-sum
      accumulator, sqrt, DVE reciprocal + scale.
    - h slices are AllGathered across the 8 cores between reps so every core
      has the full gather table.
"""

import sys

sys.path.insert(0, "/opt/trn_rl_repo")

from contextlib import ExitStack

import numpy as np

import concourse.bass as bass
import concourse.tile as tile
from concourse import bacc, mybir
from concourse import bass_utils
from concourse.library_config import mlp

P = 128
F32 = mybir.dt.float32
I16 = mybir.dt.int16

N_FULL = 100000
UNITS = 128
REPS = 5
NCORES = 8
ST_TILES = 4          # tiles per gather supertile
QMAX = 32700          # max rows addressable by int16 gather indices

TRACE = False
LAST_RESULT = None
LAST_EXEC_S = None

_CACHE = {}
_RUNNER_CACHE = {}


def _preprocess_edges(src, dst, edge_w, n, ncores):
    """Sort/pad edges by (dst tile, src quarter); build the per-core device
    arrays and the shared (SPMD-identical) chunk/gather schedule."""
    assert n % ncores == 0, (n, ncores)
    nloc_raw = n // ncores
    tiles = (nloc_raw + P - 1) // P
    nloc = tiles * P
    npad = nloc * ncores

    nq = max(1, -(-npad // QMAX))
    qrows = -(-npad // nq)  # rows per quarter slice
    assert qrows <= 32767

    e = src.shape[0]
    core = dst // nloc_raw
    local = dst - core * nloc_raw
    tile_g = local // P
    dl = (local % P).astype(np.float32)
    src_pad = ((src // nloc_raw) * nloc + (src % nloc_raw)).astype(np.int64)
    q = src_pad // qrows
    idx_local = (src_pad - q * qrows).astype(np.int32)

    # bucket = (core, tile, quarter)
    bucket = (core * tiles + tile_g) * nq + q
    order = np.argsort(bucket, kind="stable")
    s_il = idx_local[order]
    s_dl = dl[order]
    s_w = edge_w[order].astype(np.float32)
    s_bk = bucket[order]

    counts = np.bincount(bucket, minlength=ncores * tiles * nq).reshape(
        ncores, tiles, nq
    )
    # chunks per (tile, quarter), shared across cores
    c_tq = -(-counts.max(axis=0) // P)  # [tiles, nq]

    n_st = -(-tiles // ST_TILES)
    # column layout: for st: for q: for t in st: C(t,q) chunks
    colbase = np.zeros((tiles, nq), np.int64)
    ops = []  # (st, q, col_start, n_chunks, idx16_col_off)
    cur = 0
    i16off = 0
    for st in range(n_st):
        t0, t1 = st * ST_TILES, min((st + 1) * ST_TILES, tiles)
        for qq in range(nq):
            op_start = cur
            for t in range(t0, t1):
                colbase[t, qq] = cur
                cur += int(c_tq[t, qq])
            nck = cur - op_start
            if nck > 0:
                ops.append((st, qq, int(op_start), int(nck), int(i16off)))
                i16off += 8 * nck  # n/16 columns = 128*nck/16
    t_total = int(cur)
    i16_total = int(i16off)

    # per-tile ordered chunk column list
    tile_cols = []
    for t in range(tiles):
        cols = []
        for qq in range(nq):
            cols.extend(range(int(colbase[t, qq]), int(colbase[t, qq] + c_tq[t, qq])))
        tile_cols.append(cols)
    # supertile of each column (for G-tile-relative addressing)
    st_colbase = [int(colbase[st * ST_TILES, 0]) for st in range(n_st)]
    st_ncols = [
        (st_colbase[st + 1] if st + 1 < n_st else t_total) - st_colbase[st]
        for st in range(n_st)
    ]

    # edge placement
    flat_counts = counts.reshape(-1)
    group_start = np.zeros(flat_counts.size, np.int64)
    group_start[1:] = np.cumsum(flat_counts)[:-1]
    pos = np.arange(e, dtype=np.int64) - np.repeat(group_start, flat_counts)

    e_core = s_bk // (tiles * nq)
    e_tile = (s_bk // nq) % tiles
    e_q = s_bk % nq
    col = colbase[e_tile, e_q] + pos // P
    row = pos % P

    edl = np.zeros((ncores, P, t_total), np.float32)
    ew = np.zeros((ncores, P, t_total), np.float32)
    edl[e_core, row, col] = s_dl
    ew[e_core, row, col] = s_w

    # idx16 wrapped layout, per op: seq j = (col - op_start)*128 + row;
    # wrapped position [j % 16, i16_off + j // 16]
    idx16 = np.zeros((ncores, 16, i16_total), np.int16)
    opstart_of_col = np.zeros(t_total, np.int64)
    i16off_of_col = np.zeros(t_total, np.int64)
    for (_st, _q, cstart, nck, ioff) in ops:
        opstart_of_col[cstart : cstart + nck] = cstart
        i16off_of_col[cstart : cstart + nck] = ioff
    j = (col - opstart_of_col[col]) * P + row
    idx16[e_core, j % 16, i16off_of_col[col] + j // 16] = s_il.astype(np.int16)
    idx16 = np.tile(idx16, (1, 8, 1))  # [ncores, 128, i16_total]

    return dict(
        nloc_raw=nloc_raw,
        tiles=tiles,
        nloc=nloc,
        npad=npad,
        nq=nq,
        qrows=qrows,
        t_total=t_total,
        i16_total=i16_total,
        ops=tuple(ops),
        tile_cols=tuple(tuple(c) for c in tile_cols),
        st_colbase=tuple(st_colbase),
        st_ncols=tuple(st_ncols),
        edl=edl,
        ew=ew,
        idx16=idx16,
    )


def _build_program(cfg):
    tiles = cfg["tiles"]
    nloc = cfg["nloc"]
    npad = cfg["npad"]
    nq = cfg["nq"]
    qrows = cfg["qrows"]
    t_total = cfg["t_total"]
    i16_total = cfg["i16_total"]
    ops = cfg["ops"]
    tile_cols = cfg["tile_cols"]
    st_colbase = cfg["st_colbase"]
    st_ncols = cfg["st_ncols"]
    reps = cfg["reps"]
    ncores = cfg["ncores"]
    full_gru = cfg["full_gru"]
    use_bz = cfg["use_bz"]
    use_bh = cfg["use_bh"]
    n_st = len(st_colbase)
    cmax_st = max(st_ncols)

    nc = bacc.Bacc("TRN2", target_bir_lowering=False, debug=False, num_devices=ncores)

    msg_d = nc.dram_tensor("msg", [npad, P], F32, kind="ExternalInput")
    idx16_d = nc.dram_tensor("idx16", [P, i16_total], I16, kind="ExternalInput")
    edl_d = nc.dram_tensor("edl", [P, t_total], F32, kind="ExternalInput")
    ew_d = nc.dram_tensor("ew", [P, t_total], F32, kind="ExternalInput")
    weffz_d = nc.dram_tensor("weffz", [P, P], F32, kind="ExternalInput")
    weffh_d = nc.dram_tensor("weffh", [P, P], F32, kind="ExternalInput")
    iota_d = nc.dram_tensor("iota", [P, P], F32, kind="ExternalInput")
    ident_d = nc.dram_tensor("ident", [P, P], F32, kind="ExternalInput")
    nbz_d = nc.dram_tensor("nbz", [P, 1], F32, kind="ExternalInput")
    bhin_d = nc.dram_tensor("bhin", [P, 1], F32, kind="ExternalInput")
    if full_gru:
        weffr_d = nc.dram_tensor("weffr", [P, P], F32, kind="ExternalInput")
        br_d = nc.dram_tensor("br", [P, 1], F32, kind="ExternalInput")
        bhrec_d = nc.dram_tensor("bhrec", [P, 1], F32, kind="ExternalInput")

    out_d = nc.dram_tensor("out", [reps, nloc, P], F32, kind="ExternalOutput")

    n_agbuf = min(2, reps - 1) if reps > 1 else 0
    ag_in = [
        nc.dram_tensor(f"ag_in{i}", [nloc, P], F32, kind="Internal")
        for i in range(n_agbuf)
    ]
    h_full = [
        nc.dram_tensor(
            f"h_full{r}", [npad, P], F32, kind="Internal", addr_space="Shared"
        )
        for r in range(reps - 1)
    ]

    with tile.TileContext(nc) as tc, ExitStack() as ctx:
        const = ctx.enter_context(tc.tile_pool(name="const", bufs=1))
        gpool = ctx.enter_context(tc.tile_pool(name="g", bufs=2))
        selp = ctx.enter_context(tc.tile_pool(name="sel", bufs=4))
        work = ctx.enter_context(tc.tile_pool(name="work", bufs=2))
        psp = ctx.enter_context(tc.tile_pool(name="ps", bufs=2, space="PSUM"))

        nc.gpsimd.load_library(mlp)

        idx16_sb = const.tile([P, i16_total], I16)
        nc.sync.dma_start(idx16_sb[:], idx16_d[:])
        edl_sb = const.tile([P, t_total], F32)
        nc.sync.dma_start(edl_sb[:], edl_d[:])
        ew_sb = const.tile([P, t_total], F32)
        nc.sync.dma_start(ew_sb[:], ew_d[:])
        iota_sb = const.tile([P, P], F32)
        nc.sync.dma_start(iota_sb[:], iota_d[:])
        ident_sb = const.tile([P, P], F32)
        nc.sync.dma_start(ident_sb[:], ident_d[:])
        weffz_sb = const.tile([P, P], F32)
        nc.sync.dma_start(weffz_sb[:], weffz_d[:])
        weffh_sb = const.tile([P, P], F32)
        nc.sync.dma_start(weffh_sb[:], weffh_d[:])
        nbz_sb = const.tile([P, 1], F32)
        nc.sync.dma_start(nbz_sb[:], nbz_d[:])
        bhin_sb = const.tile([P, 1], F32)
        nc.sync.dma_start(bhin_sb[:], bhin_d[:])
        if full_gru:
            weffr_sb = const.tile([P, P], F32)
            nc.sync.dma_start(weffr_sb[:], weffr_d[:])
            br_sb = const.tile([P, 1], F32)
            nc.sync.dma_start(br_sb[:], br_d[:])
            bhrec_sb = const.tile([P, 1], F32)
            nc.sync.dma_start(bhrec_sb[:], bhrec_d[:])

        ngx = 3 if full_gru else 2

        for r in range(reps):
            table = msg_d if r == 0 else h_full[r - 1]
            for st in range(n_st):
                scb = st_colbase[st]
                g_sb = gpool.tile([P, cmax_st, P], F32, tag="G")
                for (op_st, qq, cstart, nck, ioff) in ops:
                    if op_st != st:
                        continue
                    qa = qq * qrows
                    qb = min(qa + qrows, npad)
                    nc.gpsimd.dma_gather(
                        g_sb[:, cstart - scb : cstart - scb + nck, :],
                        table[qa:qb, :],
                        idx16_sb[:, ioff : ioff + 8 * nck],
                        nck * P,
                        nck * P,
                        P,
                        single_packet=False,
                    )
                t0, t1 = st * ST_TILES, min((st + 1) * ST_TILES, tiles)
                for t in range(t0, t1):
                    cols = tile_cols[t]
                    pool_ps = psp.tile([P, P], F32, tag="pool")
                    for k, colg in enumerate(cols):
                        sel_sb = selp.tile([P, P], F32, tag="sel")
                        nc.vector.tensor_scalar(
                            out=sel_sb[:],
                            in0=iota_sb[:],
                            scalar1=edl_sb[:, colg : colg + 1],
                            scalar2=ew_sb[:, colg : colg + 1],
                            op0=mybir.AluOpType.is_equal,
                            op1=mybir.AluOpType.mult,
                        )
                        nc.tensor.matmul(
                            out=pool_ps[:],
                            lhsT=g_sb[:, colg - scb, :],
                            rhs=sel_sb[:],
                            start=(k == 0),
                            stop=(k == len(cols) - 1),
                        )

                    pool_sb = work.tile([P, P], F32, tag="poolsb")
                    nc.scalar.copy(pool_sb[:], pool_ps[:])

                    gx_ps = psp.tile([P, ngx * P], F32, tag="gx")
                    nc.tensor.matmul(
                        out=gx_ps[:, 0:P], lhsT=weffz_sb[:], rhs=pool_sb[:],
                        start=True, stop=True,
                    )
                    nc.tensor.matmul(
                        out=gx_ps[:, P : 2 * P], lhsT=weffh_sb[:], rhs=pool_sb[:],
                        start=True, stop=True,
                    )
                    if full_gru:
                        nc.tensor.matmul(
                            out=gx_ps[:, 2 * P : 3 * P], lhsT=weffr_sb[:],
                            rhs=pool_sb[:], start=True, stop=True,
                        )

                    omz_sb = work.tile([P, P], F32, tag="omz")
                    nc.scalar.activation(
                        out=omz_sb[:],
                        in_=gx_ps[:, 0:P],
                        func=mybir.ActivationFunctionType.Sigmoid,
                        bias=nbz_sb[:, :1] if use_bz else 0.0,
                        scale=-1.0,
                    )
                    hh_sb = work.tile([P, P], F32, tag="hh")
                    if full_gru:
                        r_sb = work.tile([P, P], F32, tag="r")
                        nc.scalar.activation(
                            out=r_sb[:],
                            in_=gx_ps[:, 2 * P : 3 * P],
                            func=mybir.ActivationFunctionType.Sigmoid,
                            bias=br_sb[:, :1],
                        )
                        rb_sb = work.tile([P, P], F32, tag="rb")
                        nc.vector.tensor_scalar(
                            out=rb_sb[:], in0=r_sb[:], scalar1=bhrec_sb[:, :1],
                            scalar2=None, op0=mybir.AluOpType.mult,
                        )
                        s_sb = work.tile([P, P], F32, tag="s")
                        nc.vector.tensor_tensor(
                            out=s_sb[:], in0=gx_ps[:, P : 2 * P], in1=rb_sb[:],
                            op=mybir.AluOpType.add,
                        )
                        nc.scalar.activation(
                            out=hh_sb[:], in_=s_sb[:],
                            func=mybir.ActivationFunctionType.Tanh,
                            bias=bhin_sb[:, :1] if use_bh else 0.0,
                        )
                    else:
                        nc.scalar.activation(
                            out=hh_sb[:], in_=gx_ps[:, P : 2 * P],
                            func=mybir.ActivationFunctionType.Tanh,
                            bias=bhin_sb[:, :1] if use_bh else 0.0,
                        )

                    comb_sb = work.tile([P, P], F32, tag="comb")
                    nc.vector.tensor_tensor(
                        out=comb_sb[:], in0=omz_sb[:], in1=hh_sb[:],
                        op=mybir.AluOpType.mult,
                    )

                    tr_ps = psp.tile([P, P], F32, tag="tr")
                    nc.tensor.transpose(
                        out=tr_ps[:], in_=comb_sb[:], identity=ident_sb[:]
                    )
                    sq_sb = work.tile([P, P], F32, tag="sq")
                    ss_sb = work.tile([P, 1], F32, tag="ss")
                    nc.scalar.activation(
                        out=sq_sb[:], in_=tr_ps[:],
                        func=mybir.ActivationFunctionType.Square,
                        accum_out=ss_sb[:],
                    )
                    nc.vector.tensor_scalar(
                        out=ss_sb[:], in0=ss_sb[:], scalar1=1e-12, scalar2=None,
                        op0=mybir.AluOpType.max,
                    )
                    nrm_sb = work.tile([P, 1], F32, tag="nrm")
                    nc.scalar.activation(
                        out=nrm_sb[:], in_=ss_sb[:],
                        func=mybir.ActivationFunctionType.Sqrt,
                    )
                    rn_sb = work.tile([P, 1], F32, tag="rn")
                    nc.vector.reciprocal(rn_sb[:], nrm_sb[:])
                    h_sb = work.tile([P, P], F32, tag="h")
                    nc.vector.tensor_scalar(
                        out=h_sb[:], in0=tr_ps[:], scalar1=rn_sb[:, :1],
                        scalar2=None, op0=mybir.AluOpType.mult,
                    )

                    nc.sync.dma_start(out_d[r, t * P : (t + 1) * P, :], h_sb[:])
                    if r < reps - 1:
                        nc.sync.dma_start(
                            ag_in[r % 2][t * P : (t + 1) * P, :], h_sb[:]
                        )
            if r < reps - 1:
                nc.gpsimd.collective_compute(
                    "AllGather",
                    mybir.AluOpType.bypass,
                    replica_groups=[list(range(ncores))],
                    ins=[ag_in[r % 2][:]],
                    outs=[h_full[r][:]],
                )

    nc.compile()
    return nc


def _prepare(message, src, dst, edge_w, W_node, b_node, W_gru, U_gru, b_in, b_rec,
             n, ncores, reps):
    pre = _preprocess_edges(
        np.asarray(src), np.asarray(dst), np.asarray(edge_w), n, ncores
    )
    nloc_raw, nloc, npad = pre["nloc_raw"], pre["nloc"], pre["npad"]

    W_node64 = np.asarray(W_node, np.float64)
    W_gru64 = np.asarray(W_gru, np.float64)
    W_eff = (W_node64 @ W_gru64).astype(np.float32)
    b_in_eff = (
        np.asarray(b_node, np.float64) @ W_gru64 + np.asarray(b_in, np.float64)
    ).astype(np.float32)
    u = W_eff.shape[0]
    assert u == P
    b_rec = np.asarray(b_rec, np.float32)
    bz = b_in_eff[0:u] + b_rec[0:u]
    br = b_in_eff[u : 2 * u] + b_rec[u : 2 * u]
    bh_in = b_in_eff[2 * u : 3 * u]
    bh_rec = b_rec[2 * u : 3 * u]
    full_gru = bool(np.any(bh_rec != 0.0))
    use_bz = bool(np.any(bz != 0.0))
    use_bh = bool(np.any(bh_in != 0.0))

    msg = np.asarray(message, np.float32)
    msg_pad = np.zeros((npad, P), np.float32)
    msg_pad.reshape(ncores, nloc, P)[:, :nloc_raw, :] = msg.reshape(
        ncores, nloc_raw, P
    )

    iota = np.tile(np.arange(P, dtype=np.float32), (P, 1))
    ident = np.eye(P, dtype=np.float32)

    cfg = dict(
        tiles=pre["tiles"],
        nloc=nloc,
        npad=npad,
        nq=pre["nq"],
        qrows=pre["qrows"],
        t_total=pre["t_total"],
        i16_total=pre["i16_total"],
        ops=pre["ops"],
        tile_cols=pre["tile_cols"],
        st_colbase=pre["st_colbase"],
        st_ncols=pre["st_ncols"],
        reps=reps,
        ncores=ncores,
        full_gru=full_gru,
        use_bz=use_bz,
        use_bh=use_bh,
    )

    in_maps = []
    for c in range(ncores):
        m = dict(
            msg=msg_pad,
            idx16=np.ascontiguousarray(pre["idx16"][c]),
            edl=np.ascontiguousarray(pre["edl"][c]),
            ew=np.ascontiguousarray(pre["ew"][c]),
            weffz=np.ascontiguousarray(W_eff[:, 0:P]),
            weffh=np.ascontiguousarray(W_eff[:, 2 * P : 3 * P]),
            iota=iota,
            ident=ident,
            nbz=(-bz).reshape(P, 1).astype(np.float32),
            bhin=bh_in.reshape(P, 1).astype(np.float32),
        )
        if full_gru:
            m["weffr"] = np.ascontiguousarray(W_eff[:, P : 2 * P])
            m["br"] = br.reshape(P, 1).astype(np.float32)
            m["bhrec"] = bh_rec.reshape(P, 1).astype(np.float32)
        in_maps.append(m)

    return cfg, in_maps, pre


def _assemble_output(res_list, cfg, n, reps):
    # Match the reference faithfully: concat states along features then
    # reshape [n, UNITS, REPS] (an interleaving reshape, NOT a stack):
    # out[n, i, j] = states[(i*reps + j) // P][n, (i*reps + j) % P]
    ncores = cfg["ncores"]
    nloc_raw = n // ncores
    parts = []
    for c in range(ncores):
        buf = res_list[c]["out"]  # [reps, nloc, P]
        parts.append(
            np.transpose(buf[:, :nloc_raw, :], (1, 0, 2)).reshape(
                nloc_raw, P, reps
            )
        )
    return np.ascontiguousarray(np.concatenate(parts, axis=0), dtype=np.float32)


def _get_runner(nc, ncores):
    """Persistent jitted executable for an SPMD Bass program (mirrors
    bass2jax.run_bass_via_pjrt's multi-core path but compiles once)."""
    key = id(nc)
    if key in _RUNNER_CACHE:
        return _RUNNER_CACHE[key]

    import jax
    from jax.sharding import Mesh, PartitionSpec
    from jax.experimental.shard_map import shard_map
    from concourse import bass2jax

    bass2jax.install_neuronx_cc_hook()
    assert nc.dbg_addr is None
    partition_name = (
        nc.partition_id_tensor.name if nc.partition_id_tensor else None
    )

    in_names, out_names, out_avals, zero_shapes = [], [], [], []
    for alloc in nc.m.functions[0].allocations:
        if not isinstance(alloc, mybir.MemoryLocationSet):
            continue
        name = alloc.memorylocations[0].name
        if alloc.kind == "ExternalInput":
            if name != partition_name:
                in_names.append(name)
        elif alloc.kind == "ExternalOutput":
            out_names.append(name)
            shape = tuple(alloc.tensor_shape)
            dtype = mybir.dt.np(alloc.dtype)
            out_avals.append(jax.core.ShapedArray(shape, dtype))
            zero_shapes.append((shape, dtype))
    n_params = len(in_names)
    all_names = in_names + out_names
    if partition_name is not None:
        all_names = all_names + [partition_name]
    donate = tuple(range(n_params, n_params + len(out_names)))

    def _body(*args):
        operands = list(args)
        if partition_name is not None:
            operands.append(bass2jax.partition_id_tensor())
        outs = bass2jax._bass_exec_p.bind(
            *operands,
            out_avals=tuple(out_avals),
            in_names=tuple(all_names),
            out_names=tuple(out_names),
            lowering_input_output_aliases=(),
            sim_require_finite=True,
            sim_require_nnan=True,
            nc=nc,
        )
        return tuple(outs)

    devices = jax.devices()[:ncores]
    assert len(devices) == ncores
    mesh = Mesh(np.asarray(devices), ("core",))
    nin = n_params + len(out_names)
    jitted = jax.jit(
        shard_map(
            _body,
            mesh=mesh,
            in_specs=(PartitionSpec("core"),) * nin,
            out_specs=(PartitionSpec("core"),) * len(out_names),
            check_rep=False,
        ),
        donate_argnums=donate,
        keep_unused=True,
    )
    runner = dict(
        jitted=jitted,
        in_names=in_names,
        out_names=out_names,
        out_avals=out_avals,
        zero_shapes=zero_shapes,
        mesh=mesh,
    )
    _RUNNER_CACHE[key] = runner
    return runner


def _execute(nc, in_maps, ncores, time_it=False):
    """Run the program; returns (results_list, exec_seconds_or_None)."""
    import time as _time

    import jax

    r = _get_runner(nc, ncores)
    concat_in = [
        np.concatenate([np.asarray(in_maps[c][name]) for c in range(ncores)], axis=0)
        for name in r["in_names"]
    ]
    zeros = [
        np.zeros((ncores * s[0], *s[1:]), d) for (s, d) in r["zero_shapes"]
    ]
    in_dev = [jax.device_put(a) for a in concat_in]
    zero_dev = [jax.device_put(z) for z in zeros]
    for a in in_dev + zero_dev:
        a.block_until_ready()
    t0 = _time.time()
    out_arrs = r["jitted"](*in_dev, *zero_dev)
    for o in out_arrs:
        o.block_until_ready()
    dt = _time.time() - t0
    results = [
        {
            name: np.asarray(out_arrs[i]).reshape(
                ncores, *r["out_avals"][i].shape
            )[c]
            for i, name in enumerate(r["out_names"])
        }
        for c in range(ncores)
    ]
    return results, dt


def bench(message, src, dst, edge_w, W_node, b_node, W_gru, U_gru, b_in, b_rec,
          iters=3):
    """Steady-state per-call wall time (s) of the compiled executable."""
    n = np.asarray(message).shape[0]
    cfg, in_maps, _pre = _prepare(
        message, src, dst, edge_w, W_node, b_node, W_gru, U_gru, b_in, b_rec,
        n, NCORES, REPS,
    )
    key = (n, cfg["t_total"], cfg["ops"], cfg["full_gru"], cfg["use_bz"],
           cfg["use_bh"], REPS, NCORES)
    nc = _CACHE.get(key)
    if nc is None:
        nc = _build_program(cfg)
        _CACHE[key] = nc
    times = []
    for _ in range(iters):
        _res, dt = _execute(nc, in_maps, NCORES)
        times.append(dt)
    return times


def kernel(message, src, dst, edge_w, W_node, b_node, W_gru, U_gru, b_in, b_rec):
    global LAST_RESULT, LAST_EXEC_S
    message = np.asarray(message)
    n = message.shape[0]
    reps = REPS
    ncores = NCORES

    cfg, in_maps, _pre = _prepare(
        message, src, dst, edge_w, W_node, b_node, W_gru, U_gru, b_in, b_rec,
        n, ncores, reps,
    )

    key = (n, cfg["t_total"], cfg["ops"], cfg["full_gru"], cfg["use_bz"],
           cfg["use_bh"], reps, ncores)
    nc = _CACHE.get(key)
    if nc is None:
        nc = _build_program(cfg)
        _CACHE[key] = nc

    results, dt = _execute(nc, in_maps, ncores)
    LAST_RESULT = None
    LAST_EXEC_S = dt
    return _assemble_output(results, cfg, n, reps)
